# revision 1
# baseline (speedup 1.0000x reference)
"""Causal self-attention (GQA + RoPE + QK-norm) Trainium2 Bass kernel.

Sharding: 8 cores = 4 batches x 2 head-groups.  Core c -> batch c//2,
q heads (c%2)*8..+8, kv heads (c%2)*2..+2.  wproj is row-sharded, so each
core emits a partial (T, C) output; the host sums the two partials per batch.

Device-side layout strategy (per core):
  - x is fed pre-transposed (xT, [C, T]) and bf16-cast by the host.
  - QKV projections produce Q,K token-major ([tok, cols]); RoPE + rms-norm
    run token-major (free-axis per-head reductions), then 128x128 PE
    transposes produce qT/kT feature-major for the attention matmuls.
    V is produced token-major, which is exactly the p@v stationary layout.
  - scores are computed transposed (scoresT[tk, tq]) so that after exp the
    p tiles are already the moving operand for the p@v matmul; the softmax
    denominator comes from a ones-column matmul accumulated in PSUM.
  - exp has no max-subtraction: qk-norm bounds |s| <= sqrt(128) ~ 11.32.
  - output projection accumulates over the 8 local heads; partial written
    fp32 to DRAM.
"""

import numpy as np
import ml_dtypes
from contextlib import ExitStack

import concourse.bass as bass
import concourse.mybir as mybir
import concourse.tile as tile
from concourse import bacc
from concourse.bass_utils import run_bass_kernel_spmd
from concourse.masks import make_identity

BF16 = mybir.dt.bfloat16
F32 = mybir.dt.float32
F32R = mybir.dt.float32r
AF = mybir.ActivationFunctionType

B, T, C = 4, 2048, 2048
H, KV, D = 16, 4, 128
HG, KVG = H // 2, KV // 2          # per-core q heads (8), kv heads (2)
QC, KC = HG * D, KVG * D           # 1024, 256
P = 128
TOKCH = T // P                     # 16 token chunks
NREP = H // KV                     # 4
EPS = 1e-5
NEG = -1.0e5                       # additive causal mask (exp -> 0)


DEBUG_DUMP = False
PHASES = ("A", "B", "C")


def _build():
    nc = bacc.Bacc("TRN2", target_bir_lowering=False, debug=False, num_devices=8)
    xt = nc.dram_tensor("xt", [C, T], BF16, kind="ExternalInput")
    wq = nc.dram_tensor("wq", [C, QC], BF16, kind="ExternalInput")
    wkv = nc.dram_tensor("wkv", [C, 2 * KC], BF16, kind="ExternalInput")
    wp = nc.dram_tensor("wp", [QC, C], BF16, kind="ExternalInput")
    cosd = nc.dram_tensor("cosd", [T, D // 2], F32, kind="ExternalInput")
    sind = nc.dram_tensor("sind", [T, D // 2], F32, kind="ExternalInput")
    out = nc.dram_tensor("out", [T, C], F32, kind="ExternalOutput")
    if DEBUG_DUMP:
        d_qt = nc.dram_tensor("d_qt", [P, HG, T], F32, kind="ExternalOutput")
        d_kt = nc.dram_tensor("d_kt", [P, KVG, T], F32, kind="ExternalOutput")
        d_v = nc.dram_tensor("d_v", [P, TOKCH, KC], F32, kind="ExternalOutput")
        d_yt = nc.dram_tensor("d_yt", [P, HG, T], F32, kind="ExternalOutput")

    with tile.TileContext(nc) as tc, ExitStack() as ctx:
        singles = ctx.enter_context(tc.tile_pool(name="singles", bufs=1))

        # ---- resident tensors ----
        wq_sb = singles.tile([P, C // P, QC], BF16)
        wkv_sb = singles.tile([P, C // P, 2 * KC], BF16)
        wqr = wq.rearrange("(co p) q -> p co q", p=P)
        wkvr = wkv.rearrange("(co p) q -> p co q", p=P)
        for co in range(C // P):
            nc.sync.dma_start(wq_sb[:, co, :], wqr[:, co, :])
            nc.sync.dma_start(wkv_sb[:, co, :], wkvr[:, co, :])
        cos_sb = singles.tile([P, TOKCH, D // 2], F32)
        nc.sync.dma_start(cos_sb, cosd.rearrange("(tc p) d -> p tc d", p=P))
        sin_sb = singles.tile([P, TOKCH, D // 2], F32)
        nc.sync.dma_start(sin_sb, sind.rearrange("(tc p) d -> p tc d", p=P))

        ident = singles.tile([P, P], BF16)
        make_identity(nc, ident)
        ones_col = singles.tile([P, 1], BF16)
        nc.vector.memset(ones_col, 1.0)
        ones_row = singles.tile([1, P], F32)
        nc.vector.memset(ones_row, 1.0)
        zero_col = singles.tile([P, 1], F32)
        nc.vector.memset(zero_col, 0.0)
        eps_col = singles.tile([P, 1], F32)
        nc.vector.memset(eps_col, EPS)
        nc.const_aps.aps[(F32, 0.0)] = zero_col[:]
        nc.const_aps.aps[(F32, EPS)] = eps_col[:]

        # 4 diagonal-block masks: variant o (offset o*128): keep where
        # i >= j + o*128  (j = tk partition, i = tq free)
        mask_sb = singles.tile([P, 4, 512], F32)
        nc.vector.memset(mask_sb, 0.0)
        for o in range(4):
            nc.gpsimd.affine_select(
                out=mask_sb[:, o, :], in_=mask_sb[:, o, :],
                compare_op=mybir.AluOpType.is_ge, fill=NEG,
                base=-o * P, pattern=[[1, 512]], channel_multiplier=-1,
            )

        qT = singles.tile([P, HG, T], BF16)      # [d, h, tok]
        kT = singles.tile([P, KVG, T], BF16)
        v_sb = singles.tile([P, TOKCH, KC], BF16)  # [tok%128, chunk, vcol]
        yT = singles.tile([P, HG, T], BF16)

        # ================= phase A: QKV proj + RoPE + qk-norm =============
        if "A" not in PHASES:
            pass
        else:
         with tc.tile_pool(name="xa", bufs=3) as xpool, \
             tc.tile_pool(name="pa", bufs=2, space="PSUM") as pps, \
             tc.tile_pool(name="sa", bufs=3) as spool:
            for t in range(TOKCH):
                xtile = xpool.tile([P, C // P, P], BF16, tag="xt")
                nc.sync.dma_start(xtile, xt.rearrange("(co p) t -> p co t", p=P)[:, :, t * P:(t + 1) * P])
                ps_q0 = pps.tile([P, 512], F32, tag="q0")
                ps_q1 = pps.tile([P, 512], F32, tag="q1")
                ps_kv = pps.tile([P, 512], F32, tag="kv")
                ps_k = ps_kv[:, 0:KC]
                ps_v = ps_kv[:, KC:2 * KC]
                nco = C // P
                for co in range(nco):
                    lhsT = xtile[:, co, :]
                    st = dict(start=(co == 0), stop=(co == nco - 1))
                    nc.tensor.matmul(ps_q0, lhsT, wq_sb[:, co, 0:512], **st)
                    nc.tensor.matmul(ps_q1, lhsT, wq_sb[:, co, 512:1024], **st)
                    nc.tensor.matmul(ps_kv, lhsT, wkv_sb[:, co, :], **st)

                # V: cast straight to resident token-major buffer
                nc.vector.tensor_copy(v_sb[:, t, :], ps_v)

                # Q/K: fused multi-head rope + rms-norm + cast + transpose
                def rope_norm(ps, nh, dstT, h0, qscale):
                    h2 = D // 2
                    v4 = ps.rearrange("p (h a d) -> p h a d", h=nh, a=2)
                    q1, q2 = v4[:, :, 0, :], v4[:, :, 1, :]
                    r = spool.tile([P, nh, 2, h2], F32, tag=f"rope{nh}")
                    r1, r2 = r[:, :, 0, :], r[:, :, 1, :]
                    s2 = spool.tile([P, nh, h2], F32, tag=f"scr{nh}")
                    cs = cos_sb[:, t, None, :].to_broadcast([P, nh, h2])
                    sn = sin_sb[:, t, None, :].to_broadcast([P, nh, h2])
                    nc.vector.tensor_mul(r1, q1, cs)
                    nc.vector.tensor_mul(s2, q2, sn)
                    nc.vector.tensor_sub(r1, r1, s2)
                    nc.vector.tensor_mul(r2, q1, sn)
                    nc.vector.tensor_mul(s2, q2, cs)
                    nc.vector.tensor_add(r2, r2, s2)
                    rf = r.rearrange("p h a d -> p h (a d)")
                    sq = spool.tile([P, nh, D], F32, tag=f"sq{nh}")
                    nc.scalar.activation(sq, rf, AF.Square)
                    ss = spool.tile([P, nh], F32, tag=f"ss{nh}")
                    nc.vector.tensor_reduce(ss, sq, axis=mybir.AxisListType.X,
                                            op=mybir.AluOpType.add)
                    rt = spool.tile([P, nh], F32, tag=f"rt{nh}")
                    nc.scalar.activation(rt, ss, AF.Sqrt, scale=1.0 / D, bias=EPS)
                    rq = spool.tile([P, nh], F32, tag=f"rq{nh}")
                    nc.vector.reciprocal(rq, rt)
                    if qscale != 1.0:
                        nc.vector.tensor_scalar_mul(rq, rq, qscale)
                    qbf = spool.tile([P, nh, D], BF16, tag=f"qbf{nh}")
                    nc.vector.tensor_mul(qbf, rf, rq[:, :, None].to_broadcast([P, nh, D]))
                    pst = pps.tile([P, 4, P], BF16, tag="tr")
                    for i in range(nh):
                        nc.tensor.transpose(pst[:, i, :], qbf[:, i, :], ident)
                    # one strided copy: psum [128, nh*128] -> nh head slices of dstT
                    nc.vector.tensor_copy(
                        dstT[:, h0:h0 + nh, t * P:(t + 1) * P], pst[:, 0:nh, :])

                qsc = 1.0 / float(np.sqrt(D))
                rope_norm(ps_q0, 4, qT, 0, qsc)
                rope_norm(ps_q1, 4, qT, 4, qsc)
                rope_norm(ps_k, KVG, kT, 0, 1.0)

        # ================= phase B: attention ============================
        if "B" not in PHASES:
            pass
        else:
         with tc.tile_pool(name="psc", bufs=4, space="PSUM") as psc, \
             tc.tile_pool(name="psy", bufs=2, space="PSUM") as psy, \
             tc.tile_pool(name="pss", bufs=2, space="PSUM") as pss, \
             tc.tile_pool(name="pb", bufs=4) as ppool, \
             tc.tile_pool(name="sb", bufs=3) as bpool:
            NT = T // 512  # 4 tq tiles
            for t in range(NT):
                for h in range(HG):
                    g = h // NREP
                    nch = 4 * (t + 1)
                    ps_y = psy.tile([P, 512], F32, tag="y")
                    ps_sden = pss.tile([P, 512], F32, tag="sden")
                    ps_s = ps_sden[0:1, :]
                    for c in range(nch):
                        o = c * P - t * 512
                        col0 = max(o, 0)
                        ps_sc = psc.tile([P, 512], F32, tag="sc")
                        nc.tensor.matmul(
                            ps_sc[:, col0:512], kT[:, g, c * P:(c + 1) * P],
                            qT[:, h, t * 512 + col0:(t + 1) * 512],
                            start=True, stop=True)
                        if o >= 0:
                            # after the col0 shift the partial block is always
                            # the i' >= j triangle
                            nc.vector.tensor_add(ps_sc[:, col0:col0 + P],
                                                 ps_sc[:, col0:col0 + P],
                                                 mask_sb[:, 0, 0:P])
                        pt = ppool.tile([P, 512], BF16, tag="pt")
                        nc.scalar.activation(pt[:, col0:512], ps_sc[:, col0:512], AF.Exp)
                        st = dict(start=(c == 0), stop=(c == nch - 1))
                        nc.tensor.matmul(ps_y[:, col0:512],
                                         v_sb[:, c, g * P:(g + 1) * P],
                                         pt[:, col0:512], **st)
                        nc.tensor.matmul(ps_s[:, col0:512], ones_col,
                                         pt[:, col0:512], **st)
                    rc = bpool.tile([1, 512], F32, tag="rc")
                    nc.vector.reciprocal(rc, ps_s)
                    nc.tensor.matmul(ps_sden, ones_row, rc, start=True, stop=True)
                    rb = bpool.tile([P, 512], F32, tag="rb")
                    nc.vector.tensor_copy(rb, ps_sden)
                    nc.vector.tensor_mul(yT[:, h, t * 512:(t + 1) * 512], ps_y, rb)

        if DEBUG_DUMP:
            with tc.tile_pool(name="dbg", bufs=2) as dpool:
                for h in range(HG):
                    dt_ = dpool.tile([P, T], F32, tag="d")
                    nc.vector.tensor_copy(dt_, qT[:, h, :])
                    nc.sync.dma_start(d_qt[:, h, :], dt_)
                    dt_ = dpool.tile([P, T], F32, tag="d")
                    nc.vector.tensor_copy(dt_, yT[:, h, :])
                    nc.sync.dma_start(d_yt[:, h, :], dt_)
                for g in range(KVG):
                    dt_ = dpool.tile([P, T], F32, tag="d")
                    nc.vector.tensor_copy(dt_, kT[:, g, :])
                    nc.sync.dma_start(d_kt[:, g, :], dt_)
                dt_ = dpool.tile([P, TOKCH * KC], F32, tag="d")
                nc.vector.tensor_copy(dt_.rearrange("p (a b) -> p a b", a=TOKCH), v_sb[:, :, :])
                nc.sync.dma_start(d_v[:, :, :], dt_.rearrange("p (a b) -> p a b", a=TOKCH))

        # ================= phase C: output projection =====================
        if "C" not in PHASES:
            pass
        else:
         with tc.tile_pool(name="wp", bufs=1) as wpool, \
             tc.tile_pool(name="po", bufs=2, space="PSUM") as pso, \
             tc.tile_pool(name="so", bufs=3) as opool:
            wpr = wp.rearrange("(hc p) c -> p hc c", p=P)
            wp_ts = []
            for ct in range(C // 512):
                wp_t = wpool.tile([P, HG, 512], BF16, tag=f"wpt{ct}")
                nc.sync.dma_start(wp_t, wpr[:, :, ct * 512:(ct + 1) * 512])
                wp_ts.append(wp_t)
            for t in range(TOKCH):
                for ct in range(C // 512):
                    ps_o = pso.tile([P, 512], F32, tag="o")
                    for hc in range(HG):
                        nc.tensor.matmul(
                            ps_o, yT[:, hc, t * P:(t + 1) * P], wp_ts[ct][:, hc, :],
                            start=(hc == 0), stop=(hc == HG - 1))
                    ob = opool.tile([P, 512], F32, tag="ob")
                    nc.vector.tensor_copy(ob, ps_o)
                    nc.sync.dma_start(out[t * P:(t + 1) * P, ct * 512:(ct + 1) * 512], ob)
    nc.compile()
    return nc


_NC_CACHE = []


def _get_prog():
    if not _NC_CACHE:
        _NC_CACHE.append(_build())
    return _NC_CACHE[0]


def _make_in_maps(inputs):
    x, cos, sin = inputs["x"], inputs["cos"], inputs["sin"]
    wq, wk, wv, wproj = inputs["wq"], inputs["wk"], inputs["wv"], inputs["wproj"]
    bf = ml_dtypes.bfloat16
    cos2 = np.ascontiguousarray(cos.reshape(T, D // 2), dtype=np.float32)
    sin2 = np.ascontiguousarray(sin.reshape(T, D // 2), dtype=np.float32)
    in_maps = []
    for core in range(8):
        b, g = core // 2, core % 2
        qs = slice(g * QC, (g + 1) * QC)
        ks = slice(g * KC, (g + 1) * KC)
        in_maps.append({
            "xt": np.ascontiguousarray(x[b].T).astype(bf),
            "wq": np.ascontiguousarray(wq[:, qs]).astype(bf),
            "wkv": np.ascontiguousarray(np.hstack([wk[:, ks], wv[:, ks]])).astype(bf),
            "wp": np.ascontiguousarray(wproj[qs, :]).astype(bf),
            "cosd": cos2,
            "sind": sin2,
        })
    return in_maps


def kernel(x, cos, sin, wq, wk, wv, wproj):
    nc = _get_prog()
    in_maps = _make_in_maps(dict(x=x, cos=cos, sin=sin, wq=wq, wk=wk, wv=wv, wproj=wproj))
    res = run_bass_kernel_spmd(nc, in_maps, core_ids=list(range(8))).results
    outp = np.empty((B, T, C), np.float32)
    for b in range(B):
        outp[b] = res[2 * b]["out"] + res[2 * b + 1]["out"]
    return outp



# revision 18
# speedup vs baseline: 1.1039x; 1.1039x over previous
"""Causal self-attention (GQA + RoPE + QK-norm) Trainium2 Bass kernel.

Sharding: 8 cores = 4 batches x 2 head-groups.  Core c -> batch c//2,
q heads (c%2)*8..+8, kv heads (c%2)*2..+2.  wproj is row-sharded, so each
core emits a partial (T, C) output; the host sums the two partials per batch.

Device-side structure (per core), single fused pipeline:
  - x fed pre-transposed (xT, [C, T]) bf16; QKV projections token-major.
  - RoPE + rms-norm token-major on DVE/ACT; Q transposed to feature-major
    via PE transposes, K via XBAR dma-transpose (keeps a PSUM bank free).
  - Attention computes transposed scores (scoresT[tk, tq]); exp on ACT;
    causal diagonal blocks zeroed post-exp by GPSIMD affine_select.
  - Softmax denominator: pt chunks accumulated into P_sum on DVE (bf16),
    one ones-column matmul per (t,h) group, reciprocal, and a GPSIMD
    partition_broadcast for the normalize multiply (no fp32-moving PE
    broadcast matmul).
  - Output projection accumulates the 8 local heads; partial written fp32.
  - A/B/C phases are interleaved in emission order so the PE always has
    projection/out-proj matmuls to run while attention waits on exp, and
    a warm-up transpose stream keeps the PE p-state ramp warm during the
    initial weight DMAs.

PSUM budget (8 banks): q[128,1024]f32 x1 (2) + kv[128,512]f32 x1 (1) +
tr[128,8,128]bf16 x1 (1) + sc[128,512]f32 x2 (2) + y[128,512]f32 x2 (2).
"""

import numpy as np
import ml_dtypes
from contextlib import ExitStack

import concourse.bass as bass
import concourse.mybir as mybir
import concourse.tile as tile
from concourse import bacc
from concourse.bass_utils import run_bass_kernel_spmd
from concourse.masks import make_identity

BF16 = mybir.dt.bfloat16
F32 = mybir.dt.float32
AF = mybir.ActivationFunctionType

B, T, C = 4, 2048, 2048
H, KV, D = 16, 4, 128
HG, KVG = H // 2, KV // 2          # per-core q heads (8), kv heads (2)
QC, KC = HG * D, KVG * D           # 1024, 256
P = 128
TOKCH = T // P                     # 16 token chunks
NREP = H // KV                     # 4
EPS = 1e-5
WARMUP_N = 96                      # PE warm-up transposes during initial DMAs


def _build():
    nc = bacc.Bacc("TRN2", target_bir_lowering=False, debug=False, num_devices=8)
    xt = nc.dram_tensor("xt", [C, T], BF16, kind="ExternalInput")
    wq = nc.dram_tensor("wq", [C, QC], BF16, kind="ExternalInput")
    wkv = nc.dram_tensor("wkv", [C, 2 * KC], BF16, kind="ExternalInput")
    wp = nc.dram_tensor("wp", [QC, C], BF16, kind="ExternalInput")
    cosd = nc.dram_tensor("cosd", [T, D // 2], BF16, kind="ExternalInput")
    sind = nc.dram_tensor("sind", [T, D // 2], BF16, kind="ExternalInput")
    out = nc.dram_tensor("out", [T, C], F32, kind="ExternalOutput")

    with tile.TileContext(nc) as tc, ExitStack() as ctx:
        singles = ctx.enter_context(tc.tile_pool(name="singles", bufs=1))

        # ---- small constants first (cheap engine work, fills DMA wait) ----
        ident = singles.tile([P, P], BF16)
        make_identity(nc, ident)
        ones_col = singles.tile([P, 1], BF16)
        nc.vector.memset(ones_col, 1.0)
        zero_col = singles.tile([P, 1], F32)
        nc.vector.memset(zero_col, 0.0)
        nc.const_aps.aps[(F32, 0.0)] = zero_col[:]
        qsc = 1.0 / float(np.sqrt(D))

        # ---- resident tensors ----
        wq_sb = singles.tile([P, C // P, QC], BF16)
        wkv_sb = singles.tile([P, C // P, 2 * KC], BF16)
        qT = singles.tile([P, HG, T], BF16)      # [d, h, tok]
        kT = singles.tile([P, KVG, T], BF16)
        v_sb = singles.tile([P, TOKCH, KC], BF16)  # [tok%128, chunk, vcol]
        yT = singles.tile([P, HG, T], BF16)
        wp_sb = singles.tile([P, C // 512, HG, 512], BF16)

        # ---- pools ----
        pps = ctx.enter_context(tc.tile_pool(name="ps", bufs=1, space="PSUM"))
        xpool = ctx.enter_context(tc.tile_pool(name="xa", bufs=2))
        spool = ctx.enter_context(tc.tile_pool(name="sa", bufs=2))
        ptpool = ctx.enter_context(tc.tile_pool(name="pb", bufs=3))
        pspool = ctx.enter_context(tc.tile_pool(name="psum_sb", bufs=2))
        rpool = ctx.enter_context(tc.tile_pool(name="rp", bufs=1))
        opool = ctx.enter_context(tc.tile_pool(name="op", bufs=2))

        # ---- DMA prefetch: x tiles first, then co-interleaved weights ----
        xtiles = []
        xtr = xt.rearrange("(co p) t -> p co t", p=P)
        for j in range(2):
            xtile = xpool.tile([P, C // P, P], BF16, tag="xt", name=f"xt{j}")
            nc.sync.dma_start(xtile, xtr[:, :, j * P:(j + 1) * P])
            xtiles.append(xtile)
        # bf16 rope tables keep every rope DVE op in the 2x packed mode
        cosb = singles.tile([P, TOKCH, D // 2], BF16)
        nc.scalar.dma_start(cosb, cosd.rearrange("(tc p) d -> p tc d", p=P))
        sinb = singles.tile([P, TOKCH, D // 2], BF16)
        nc.scalar.dma_start(sinb, sind.rearrange("(tc p) d -> p tc d", p=P))
        wqr = wq.rearrange("(co p) q -> p co q", p=P)
        wkvr = wkv.rearrange("(co p) q -> p co q", p=P)
        for co in range(C // P):
            nc.sync.dma_start(wq_sb[:, co, :], wqr[:, co, :])
            nc.sync.dma_start(wkv_sb[:, co, :], wkvr[:, co, :])

        # ---- PE warm-up: keep the p-state ramp warm during DMA wait ----
        tr_ps = pps.tile([P, HG, P], BF16, tag="tr", bufs=1)
        for i in range(WARMUP_N):
            nc.tensor.transpose(tr_ps[:, i % HG, :], ident, ident)

        # ================= phase A chunk =================================
        def emit_A(j):
            # prefetch the x tile 3 chunks ahead
            if j + 2 < TOKCH:
                xtile_n = xpool.tile([P, C // P, P], BF16, tag="xt", name=f"xt{j+2}")
                nc.sync.dma_start(xtile_n, xtr[:, :, (j + 2) * P:(j + 3) * P])
                xtiles.append(xtile_n)
            xtile = xtiles[j]
            ps_q = pps.tile([P, 2 * 512], F32, tag="q", bufs=1, name=f"psq{j}")
            ps_kv = pps.tile([P, 512], F32, tag="kv", bufs=1, name=f"pskv{j}")
            nco = C // P
            # kv first: its psum (and the K rope that reads it) frees early,
            # so the next chunk's kv matmuls never head-block the PE queue.
            # ldweights re-loads from loop splitting are free (not modeled).
            for co in range(nco):
                st = dict(start=(co == 0), stop=(co == nco - 1))
                nc.tensor.matmul(ps_kv, xtile[:, co, :], wkv_sb[:, co, :], **st)
                if j < 2:
                    # first chunks are DMA-arrival-paced; keep the PE p-state
                    # ramp warm through the per-co weight-wait gaps
                    for _ in range(3):
                        nc.tensor.transpose(tr_ps[:, 7, :], ident, ident)
            for co in range(nco):
                st = dict(start=(co == 0), stop=(co == nco - 1))
                nc.tensor.matmul(ps_q[:, 0:512], xtile[:, co, :],
                                 wq_sb[:, co, 0:512], **st)
            for co in range(nco):
                st = dict(start=(co == 0), stop=(co == nco - 1))
                nc.tensor.matmul(ps_q[:, 512:1024], xtile[:, co, :],
                                 wq_sb[:, co, 512:1024], **st)

            # Evacuate PSUM immediately via ACT copies (frees the q/kv banks
            # within ~1us of the matmuls) and run rope entirely in bf16 on
            # SBUF so every DVE op hits the 2x packed mode.
            nc.scalar.activation(v_sb[:, j, :], ps_kv[:, KC:2 * KC], AF.Copy)
            ksb = spool.tile([P, KVG, 2, D // 2], BF16, tag="ksb")
            nc.scalar.activation(ksb, ps_kv[:, 0:KC].rearrange(
                "p (h a d) -> p h a d", h=KVG, a=2), AF.Copy)
            qsb = spool.tile([P, 2 * HG, D // 2], BF16, tag="qsb")
            nc.scalar.activation(qsb, ps_q, AF.Copy)

            def rope6(src, nh):
                # src [P, nh, 2, h2] bf16 -> rotated r [P, nh, 2, h2] bf16
                h2 = D // 2
                q1, q2 = src[:, :, 0, :], src[:, :, 1, :]
                r = spool.tile([P, nh, 2, h2], BF16, tag=f"rope{nh}")
                r1, r2 = r[:, :, 0, :], r[:, :, 1, :]
                s2 = spool.tile([P, nh, h2], BF16, tag=f"scr{nh}")
                cs = cosb[:, j, None, :].to_broadcast([P, nh, h2])
                sn = sinb[:, j, None, :].to_broadcast([P, nh, h2])
                nc.vector.tensor_mul(r1, q1, cs)
                nc.vector.tensor_mul(s2, q2, sn)
                nc.vector.tensor_sub(r1, r1, s2)
                nc.vector.tensor_mul(r2, q1, sn)
                nc.vector.tensor_mul(s2, q2, cs)
                nc.vector.tensor_add(r2, r2, s2)
                return r.rearrange("p h a d -> p h (a d)")

            # sum-of-squares per head into one batched tile (Pool reduces)
            ss10 = spool.tile([P, 10], F32, tag="ss10")
            rk = rope6(ksb, KVG)
            sqk = spool.tile([P, KVG, D], F32, tag="sqk")
            nc.scalar.activation(sqk, rk, AF.Square)
            nc.vector.tensor_reduce(ss10[:, 0:2], sqk, axis=mybir.AxisListType.X,
                                    op=mybir.AluOpType.add)
            rq0 = rope6(qsb.rearrange("p (h a) d -> p h a d", a=2)[:, 0:4], 4)
            sq0 = spool.tile([P, 4, D], F32, tag="sqq")
            nc.scalar.activation(sq0, rq0, AF.Square)
            nc.vector.tensor_reduce(ss10[:, 2:6], sq0, axis=mybir.AxisListType.X,
                                    op=mybir.AluOpType.add)
            rq1 = rope6(qsb.rearrange("p (h a) d -> p h a d", a=2)[:, 4:8], 4)
            sq1 = spool.tile([P, 4, D], F32, tag="sqq")
            nc.scalar.activation(sq1, rq1, AF.Square)
            nc.vector.tensor_reduce(ss10[:, 6:10], sq1, axis=mybir.AxisListType.X,
                                    op=mybir.AluOpType.add)

            # batched rsqrt on DVE: magic-constant seed + one Newton step
            # (ACT Sqrt would thrash the activation table against Exp).
            # rq = 1/sqrt(ss + D*eps); per-slice scale folded into the
            # bf16 cast below (k: sqrt(D); q: qsc*sqrt(D) == 1).
            u = spool.tile([P, 10], F32, tag="u10")
            nc.vector.tensor_scalar_add(u, ss10, D * EPS)
            rr = spool.tile([P, 10], F32, tag="rr10")
            zt = spool.tile([P, 10], F32, tag="zt10")
            I32 = mybir.dt.int32
            nc.vector.tensor_scalar(rr.bitcast(I32), u.bitcast(I32), 1, None,
                                    mybir.AluOpType.logical_shift_right)
            nc.vector.tensor_scalar(rr.bitcast(I32), rr.bitcast(I32),
                                    -1, 0x5F3759DF,
                                    mybir.AluOpType.mult, mybir.AluOpType.add)
            nc.vector.tensor_mul(zt, rr, rr)
            nc.vector.tensor_mul(zt, zt, u)
            nc.vector.tensor_scalar(zt, zt, -0.5, 1.5,
                                    mybir.AluOpType.mult, mybir.AluOpType.add)
            nc.vector.tensor_mul(rr, rr, zt)
            # per-slice scale + bf16 cast (k: sqrt(D); q: qsc*sqrt(D) == 1)
            rqb = spool.tile([P, 10], BF16, tag="rqb")
            nc.vector.tensor_scalar(rqb[:, 0:2], rr[:, 0:2],
                                    float(np.sqrt(D)), None, mybir.AluOpType.mult)
            nc.vector.tensor_copy(rqb[:, 2:10], rr[:, 2:10])

            # normalize + transpose
            kbf = spool.tile([P, KVG, D], BF16, tag="kbf", bufs=2)
            nc.vector.tensor_mul(kbf, rk, rqb[:, 0:2, None].to_broadcast([P, KVG, D]))
            for g in range(KVG):
                nc.scalar.dma_start_transpose(
                    kT[:, g, j * P:(j + 1) * P], kbf[:, g, :])
            qbf0 = spool.tile([P, 4, D], BF16, tag="qbf")
            nc.vector.tensor_mul(qbf0, rq0, rqb[:, 2:6, None].to_broadcast([P, 4, D]))
            for i in range(4):
                nc.tensor.transpose(tr_ps[:, i, :], qbf0[:, i, :], ident)
            qbf1 = spool.tile([P, 4, D], BF16, tag="qbf")
            nc.vector.tensor_mul(qbf1, rq1, rqb[:, 6:10, None].to_broadcast([P, 4, D]))
            for i in range(4):
                nc.tensor.transpose(tr_ps[:, 4 + i, :], qbf1[:, i, :], ident)
            # one strided copy: psum -> 8 head slices of qT (ACT has slack here)
            nc.scalar.activation(qT[:, :, j * P:(j + 1) * P], tr_ps, AF.Copy)

        # ================= phase B group =================================
        # The denominator tail of group g is emitted after the first chunk
        # of group g+1 so its PE ones-matmul never head-blocks the queue
        # waiting for group g's last P_sum accumulation on DVE.
        pending_tail = []

        def flush_tail():
            while pending_tail:
                pending_tail.pop(0)()

        def emit_B(t, h):
            g = h // NREP
            nch = 4 * (t + 1)
            ps_y = pps.tile([P, 512], F32, tag="y", bufs=2, name=f"psy{t}_{h}")
            p_sum = pspool.tile([P, 512], BF16, tag="ps")
            for c in range(nch):
                o = c * P - t * 512
                col0 = max(o, 0)
                ps_sc = pps.tile([P, 512], F32, tag="sc", bufs=2, name=f"sc{t}_{h}_{c}")
                nc.tensor.matmul(
                    ps_sc[:, col0:512], kT[:, g, c * P:(c + 1) * P],
                    qT[:, h, t * 512 + col0:(t + 1) * 512],
                    start=True, stop=True)
                pt = ptpool.tile([P, 512], BF16, tag="pt")
                nc.scalar.activation(pt[:, col0:512], ps_sc[:, col0:512], AF.Exp)
                if o >= 0:
                    # zero the upper (non-causal) triangle of the diag block
                    nc.gpsimd.affine_select(
                        out=pt[:, col0:col0 + P], in_=pt[:, col0:col0 + P],
                        compare_op=mybir.AluOpType.is_ge, fill=0.0,
                        base=0, pattern=[[1, P]], channel_multiplier=-1)
                if c == 0:
                    nc.vector.tensor_copy(p_sum, pt)
                else:
                    nc.vector.tensor_add(p_sum[:, col0:512], p_sum[:, col0:512],
                                         pt[:, col0:512])
                st = dict(start=(c == 0), stop=(c == nch - 1))
                nc.tensor.matmul(ps_y[:, col0:512],
                                 v_sb[:, c, g * P:(g + 1) * P],
                                 pt[:, col0:512], **st)
                if c == 0:
                    flush_tail()

            def tail():
                # denominator: one ones-matmul, reciprocal, Pool broadcast
                ps_den = pps.tile([P, 512], F32, tag="sc", bufs=2,
                                  name=f"den{t}_{h}")
                nc.tensor.matmul(ps_den[0:1, :], ones_col, p_sum,
                                 start=True, stop=True)
                rc = rpool.tile([1, 512], F32, tag="rc")
                nc.vector.reciprocal(rc, ps_den[0:1, :])
                rden = rpool.tile([P, 512], F32, tag="rd")
                nc.gpsimd.partition_broadcast(rden, rc)
                nc.vector.tensor_mul(yT[:, h, t * 512:(t + 1) * 512], ps_y, rden)

            pending_tail.append(tail)

        # ================= phase C chunk =================================
        # ps_o rides the sc ring (not y) so a deferred B tail can never
        # interleave an open ps_y accumulation between two C allocations.
        def emit_C(j):
            for ct in range(C // 512):
                ps_o = pps.tile([P, 512], F32, tag="sc", bufs=2, name=f"pso{j}_{ct}")
                for hc in range(HG):
                    nc.tensor.matmul(
                        ps_o, yT[:, hc, j * P:(j + 1) * P], wp_sb[:, ct, hc, :],
                        start=(hc == 0), stop=(hc == HG - 1))
                ob = opool.tile([P, 512], F32, tag="ob")
                nc.vector.tensor_copy(ob, ps_o)
                nc.sync.dma_start(out[j * P:(j + 1) * P, ct * 512:(ct + 1) * 512], ob)

        # ================= interleaved emission ==========================
        for j in range(4):
            emit_A(j)
        # wp weights: needed from the first C chunk (~mid-kernel)
        wpr = wp.rearrange("(hc p) (ct cc) -> p ct hc cc", p=P, cc=512)
        for ct in range(C // 512):
            nc.scalar.dma_start(wp_sb[:, ct, :, :], wpr[:, ct, :, :])

        # B(t) interleaved with remaining A chunks and C chunks:
        #   B(0) + A(4..7);  B(1) + A(8..11) + C(0..3);
        #   B(2) + A(12..15) + C(4..7);  B(3) + C(8..11);  tail C(12..15)
        fill = {
            0: {1: [("A", 4)], 3: [("A", 5)], 5: [("A", 6)], 7: [("A", 7)]},
            1: {1: [("A", 8), ("C", 0)], 3: [("A", 9), ("C", 1)],
                5: [("A", 10), ("C", 2)], 7: [("A", 11), ("C", 3)]},
            2: {1: [("A", 12), ("C", 4)], 3: [("A", 13), ("C", 5)],
                5: [("A", 14), ("C", 6)], 7: [("A", 15), ("C", 7)]},
            3: {1: [("C", 8)], 3: [("C", 9)], 5: [("C", 10)], 7: [("C", 11)]},
        }
        for t in range(4):
            for h in range(HG):
                emit_B(t, h)
                for kind, idx in fill[t].get(h, []):
                    (emit_A if kind == "A" else emit_C)(idx)
        flush_tail()
        for j in range(12, 16):
            emit_C(j)

    nc.compile()
    return nc


_NC_CACHE = []


def _get_prog():
    if not _NC_CACHE:
        _NC_CACHE.append(_build())
    return _NC_CACHE[0]


def _make_in_maps(inputs):
    x, cos, sin = inputs["x"], inputs["cos"], inputs["sin"]
    wq, wk, wv, wproj = inputs["wq"], inputs["wk"], inputs["wv"], inputs["wproj"]
    bf = ml_dtypes.bfloat16
    cos2 = np.ascontiguousarray(cos.reshape(T, D // 2)).astype(bf)
    sin2 = np.ascontiguousarray(sin.reshape(T, D // 2)).astype(bf)
    in_maps = []
    for core in range(8):
        b, g = core // 2, core % 2
        qs = slice(g * QC, (g + 1) * QC)
        ks = slice(g * KC, (g + 1) * KC)
        in_maps.append({
            "xt": np.ascontiguousarray(x[b].T).astype(bf),
            "wq": np.ascontiguousarray(wq[:, qs]).astype(bf),
            "wkv": np.ascontiguousarray(np.hstack([wk[:, ks], wv[:, ks]])).astype(bf),
            "wp": np.ascontiguousarray(wproj[qs, :]).astype(bf),
            "cosd": cos2,
            "sind": sin2,
        })
    return in_maps


def kernel(x, cos, sin, wq, wk, wv, wproj):
    nc = _get_prog()
    in_maps = _make_in_maps(dict(x=x, cos=cos, sin=sin, wq=wq, wk=wk, wv=wv, wproj=wproj))
    res = run_bass_kernel_spmd(nc, in_maps, core_ids=list(range(8))).results
    outp = np.empty((B, T, C), np.float32)
    for b in range(B):
        outp[b] = res[2 * b]["out"] + res[2 * b + 1]["out"]
    return outp


# revision 32
# speedup vs baseline: 1.1848x; 1.0733x over previous
"""Causal self-attention (GQA + RoPE + QK-norm) Trainium2 Bass kernel.

Sharding: 8 cores = 4 batches x 2 head-groups.  Core c -> batch c//2,
q heads (c%2)*8..+8, kv heads (c%2)*2..+2.  wproj is row-sharded, so each
core emits a partial (T, C) output; the host sums the two partials per batch.

Device-side structure (per core), single fused pipeline:
  - x fed pre-transposed (xT, [C, T]) bf16; QKV projections token-major.
  - RoPE + rms-norm token-major on DVE/ACT; Q transposed to feature-major
    via PE transposes, K via XBAR dma-transpose (keeps a PSUM bank free).
  - Attention computes transposed scores (scoresT[tk, tq]); exp on ACT;
    causal diagonal blocks zeroed post-exp by GPSIMD affine_select.
  - Softmax denominator: pt chunks accumulated into P_sum on DVE (bf16),
    one ones-column matmul per (t,h) group, reciprocal, and a GPSIMD
    partition_broadcast for the normalize multiply (no fp32-moving PE
    broadcast matmul).
  - Output projection accumulates the 8 local heads; partial written fp32.
  - A/B/C phases are interleaved in emission order so the PE always has
    projection/out-proj matmuls to run while attention waits on exp, and
    a warm-up transpose stream keeps the PE p-state ramp warm during the
    initial weight DMAs.

PSUM budget (8 banks): q[128,1024]f32 x1 (2) + kv[128,512]f32 x1 (1) +
tr[128,8,128]bf16 x1 (1) + sc[128,512]f32 x2 (2) + y[128,512]f32 x2 (2).
"""

import numpy as np
import ml_dtypes
from contextlib import ExitStack

import concourse.bass as bass
import concourse.mybir as mybir
import concourse.tile as tile
from concourse import bacc
from concourse.bass_utils import run_bass_kernel_spmd
from concourse.masks import make_identity

BF16 = mybir.dt.bfloat16
F32 = mybir.dt.float32
AF = mybir.ActivationFunctionType

B, T, C = 4, 2048, 2048
H, KV, D = 16, 4, 128
HG, KVG = H // 2, KV // 2          # per-core q heads (8), kv heads (2)
QC, KC = HG * D, KVG * D           # 1024, 256
P = 128
TOKCH = T // P                     # 16 token chunks
NREP = H // KV                     # 4
EPS = 1e-5
WARMUP_N = 96                      # PE warm-up transposes during initial DMAs


def _build():
    nc = bacc.Bacc("TRN2", target_bir_lowering=False, debug=False, num_devices=8)
    xt = nc.dram_tensor("xt", [C, T], BF16, kind="ExternalInput")
    wq = nc.dram_tensor("wq", [C, QC], BF16, kind="ExternalInput")
    wkv = nc.dram_tensor("wkv", [C, 2 * KC], BF16, kind="ExternalInput")
    wp = nc.dram_tensor("wp", [QC, C], BF16, kind="ExternalInput")
    cosd = nc.dram_tensor("cosd", [T, D // 2], BF16, kind="ExternalInput")
    sind = nc.dram_tensor("sind", [T, D // 2], BF16, kind="ExternalInput")
    out = nc.dram_tensor("out", [T, C], F32, kind="ExternalOutput")

    with tile.TileContext(nc) as tc, ExitStack() as ctx:
        singles = ctx.enter_context(tc.tile_pool(name="singles", bufs=1))

        # ---- small constants first (cheap engine work, fills DMA wait) ----
        ident = singles.tile([P, P], BF16)
        make_identity(nc, ident)
        ones_col = singles.tile([P, 1], BF16)
        nc.vector.memset(ones_col, 1.0)
        zero_col = singles.tile([P, 1], F32)
        nc.vector.memset(zero_col, 0.0)
        nc.const_aps.aps[(F32, 0.0)] = zero_col[:]
        qsc = 1.0 / float(np.sqrt(D))

        # ---- resident tensors ----
        wq_sb = singles.tile([P, C // P, QC], BF16)
        wkv_sb = singles.tile([P, C // P, 2 * KC], BF16)
        qT = singles.tile([P, HG, T], BF16)      # [d, h, tok]
        kT = singles.tile([P, KVG, T], BF16)
        v_sb = singles.tile([P, TOKCH, KC], BF16)  # [tok%128, chunk, vcol]
        yT = singles.tile([P, HG, T], BF16)
        wp_sb = singles.tile([P, C // 512, HG, 512], BF16)

        # ---- pools ----
        pps = ctx.enter_context(tc.tile_pool(name="ps", bufs=1, space="PSUM"))
        xpool = ctx.enter_context(tc.tile_pool(name="xa", bufs=2))
        spool = ctx.enter_context(tc.tile_pool(name="sa", bufs=2))
        ptpool = ctx.enter_context(tc.tile_pool(name="pb", bufs=3))
        pspool = ctx.enter_context(tc.tile_pool(name="psum_sb", bufs=2))
        rpool = ctx.enter_context(tc.tile_pool(name="rp", bufs=1))
        opool = ctx.enter_context(tc.tile_pool(name="op", bufs=2))

        # ---- DMA prefetch: x tiles first, then co-interleaved weights ----
        xtiles = []
        xtr = xt.rearrange("(co p) t -> p co t", p=P)
        for j in range(2):
            xtile = xpool.tile([P, C // P, P], BF16, tag="xt", name=f"xt{j}")
            nc.sync.dma_start(xtile, xtr[:, :, j * P:(j + 1) * P])
            xtiles.append(xtile)
        # bf16 rope tables keep every rope DVE op in the 2x packed mode
        cosb = singles.tile([P, TOKCH, D // 2], BF16)
        nc.scalar.dma_start(cosb, cosd.rearrange("(tc p) d -> p tc d", p=P))
        sinb = singles.tile([P, TOKCH, D // 2], BF16)
        nc.scalar.dma_start(sinb, sind.rearrange("(tc p) d -> p tc d", p=P))
        wqr = wq.rearrange("(co p) q -> p co q", p=P)
        wkvr = wkv.rearrange("(co p) q -> p co q", p=P)
        for co in range(C // P):
            nc.sync.dma_start(wq_sb[:, co, :], wqr[:, co, :])
            nc.sync.dma_start(wkv_sb[:, co, :], wkvr[:, co, :])

        # ---- PE warm-up: keep the p-state ramp warm during DMA wait ----
        tr_ps = pps.tile([P, HG, P], BF16, tag="tr", bufs=1)
        for i in range(WARMUP_N):
            nc.tensor.transpose(tr_ps[:, i % HG, :], ident, ident)

        # ================= phase A chunk =================================
        def emit_A(j):
            # prefetch the x tile 3 chunks ahead
            if j + 2 < TOKCH:
                xtile_n = xpool.tile([P, C // P, P], BF16, tag="xt", name=f"xt{j+2}")
                nc.sync.dma_start(xtile_n, xtr[:, :, (j + 2) * P:(j + 3) * P])
                xtiles.append(xtile_n)
            xtile = xtiles[j]
            ps_q = pps.tile([P, 2 * 512], F32, tag="q", bufs=1, name=f"psq{j}")
            ps_kv = pps.tile([P, 512], F32, tag="kv", bufs=1, name=f"pskv{j}")
            nco = C // P
            # kv first: its psum (and the K rope that reads it) frees early,
            # so the next chunk's kv matmuls never head-block the PE queue.
            # ldweights re-loads from loop splitting are free (not modeled).
            for co in range(nco):
                st = dict(start=(co == 0), stop=(co == nco - 1))
                nc.tensor.matmul(ps_kv, xtile[:, co, :], wkv_sb[:, co, :], **st)
                if j < 2:
                    # first chunks are DMA-arrival-paced; keep the PE p-state
                    # ramp warm through the per-co weight-wait gaps
                    for _ in range(8 if j == 0 else 3):
                        nc.tensor.transpose(tr_ps[:, 7, :], ident, ident)
            for co in range(nco):
                st = dict(start=(co == 0), stop=(co == nco - 1))
                nc.tensor.matmul(ps_q[:, 0:512], xtile[:, co, :],
                                 wq_sb[:, co, 0:512], **st)
            for co in range(nco):
                st = dict(start=(co == 0), stop=(co == nco - 1))
                nc.tensor.matmul(ps_q[:, 512:1024], xtile[:, co, :],
                                 wq_sb[:, co, 512:1024], **st)

            # Evacuate PSUM immediately via ACT copies (frees the q/kv banks
            # within ~1us of the matmuls) and run rope entirely in bf16 on
            # SBUF so every DVE op hits the 2x packed mode.
            nc.scalar.activation(v_sb[:, j, :], ps_kv[:, KC:2 * KC], AF.Copy)
            ksb = spool.tile([P, KVG, 2, D // 2], BF16, tag="ksb")
            nc.scalar.activation(ksb, ps_kv[:, 0:KC].rearrange(
                "p (h a d) -> p h a d", h=KVG, a=2), AF.Copy)
            qsb = spool.tile([P, 2 * HG, D // 2], BF16, tag="qsb")
            nc.scalar.activation(qsb, ps_q, AF.Copy)

            def rope6(src, nh):
                # src [P, nh, 2, h2] bf16 -> rotated r [P, nh, 2, h2] bf16
                h2 = D // 2
                q1, q2 = src[:, :, 0, :], src[:, :, 1, :]
                r = spool.tile([P, nh, 2, h2], BF16, tag=f"rope{nh}")
                r1, r2 = r[:, :, 0, :], r[:, :, 1, :]
                s2 = spool.tile([P, nh, h2], BF16, tag=f"scr{nh}")
                cs = cosb[:, j, None, :].to_broadcast([P, nh, h2])
                sn = sinb[:, j, None, :].to_broadcast([P, nh, h2])
                nc.vector.tensor_mul(r1, q1, cs)
                nc.vector.tensor_mul(s2, q2, sn)
                nc.vector.tensor_sub(r1, r1, s2)
                nc.vector.tensor_mul(r2, q1, sn)
                nc.vector.tensor_mul(s2, q2, cs)
                nc.vector.tensor_add(r2, r2, s2)
                return r.rearrange("p h a d -> p h (a d)")

            # sum-of-squares per head into one batched tile (Pool reduces)
            ss10 = spool.tile([P, 10], F32, tag="ss10")
            rk = rope6(ksb, KVG)
            sqk = spool.tile([P, KVG, D], F32, tag="sqk")
            nc.scalar.activation(sqk, rk, AF.Square)
            nc.vector.tensor_reduce(ss10[:, 0:2], sqk, axis=mybir.AxisListType.X,
                                    op=mybir.AluOpType.add)
            rq0 = rope6(qsb.rearrange("p (h a) d -> p h a d", a=2)[:, 0:4], 4)
            sq0 = spool.tile([P, 4, D], F32, tag="sqq")
            nc.scalar.activation(sq0, rq0, AF.Square)
            nc.vector.tensor_reduce(ss10[:, 2:6], sq0, axis=mybir.AxisListType.X,
                                    op=mybir.AluOpType.add)
            rq1 = rope6(qsb.rearrange("p (h a) d -> p h a d", a=2)[:, 4:8], 4)
            sq1 = spool.tile([P, 4, D], F32, tag="sqq")
            nc.scalar.activation(sq1, rq1, AF.Square)
            nc.vector.tensor_reduce(ss10[:, 6:10], sq1, axis=mybir.AxisListType.X,
                                    op=mybir.AluOpType.add)

            # batched rsqrt on DVE: magic-constant seed + one Newton step
            # (ACT Sqrt would thrash the activation table against Exp).
            # rq = 1/sqrt(ss + D*eps); per-slice scale folded into the
            # bf16 cast below (k: sqrt(D); q: qsc*sqrt(D) == 1).
            u = spool.tile([P, 10], F32, tag="u10")
            nc.vector.tensor_scalar_add(u, ss10, D * EPS)
            rr = spool.tile([P, 10], F32, tag="rr10")
            zt = spool.tile([P, 10], F32, tag="zt10")
            I32 = mybir.dt.int32
            nc.vector.tensor_scalar(rr.bitcast(I32), u.bitcast(I32), 1, None,
                                    mybir.AluOpType.logical_shift_right)
            nc.vector.tensor_scalar(rr.bitcast(I32), rr.bitcast(I32),
                                    -1, 0x5F3759DF,
                                    mybir.AluOpType.mult, mybir.AluOpType.add)
            nc.vector.tensor_mul(zt, rr, rr)
            nc.vector.tensor_mul(zt, zt, u)
            nc.vector.tensor_scalar(zt, zt, -0.5, 1.5,
                                    mybir.AluOpType.mult, mybir.AluOpType.add)
            nc.vector.tensor_mul(rr, rr, zt)
            # per-slice scale + bf16 cast (k: sqrt(D); q: qsc*sqrt(D) == 1)
            rqb = spool.tile([P, 10], BF16, tag="rqb")
            nc.vector.tensor_scalar(rqb[:, 0:2], rr[:, 0:2],
                                    float(np.sqrt(D)), None, mybir.AluOpType.mult)
            nc.vector.tensor_copy(rqb[:, 2:10], rr[:, 2:10])

            # normalize + transpose
            kbf = spool.tile([P, KVG, D], BF16, tag="kbf", bufs=2)
            nc.vector.tensor_mul(kbf, rk, rqb[:, 0:2, None].to_broadcast([P, KVG, D]))
            for g in range(KVG):
                nc.sync.dma_start_transpose(
                    kT[:, g, j * P:(j + 1) * P], kbf[:, g, :])
            qbf0 = spool.tile([P, 4, D], BF16, tag="qbf")
            nc.vector.tensor_mul(qbf0, rq0, rqb[:, 2:6, None].to_broadcast([P, 4, D]))
            for i in range(4):
                nc.tensor.transpose(tr_ps[:, i, :], qbf0[:, i, :], ident)
            qbf1 = spool.tile([P, 4, D], BF16, tag="qbf")
            nc.vector.tensor_mul(qbf1, rq1, rqb[:, 6:10, None].to_broadcast([P, 4, D]))
            for i in range(4):
                nc.tensor.transpose(tr_ps[:, 4 + i, :], qbf1[:, i, :], ident)
            # one strided copy: psum -> 8 head slices of qT.  On DVE: the
            # ACT queue is exp-saturated in B regions and would delay this
            # past the point the next B tile's scores need it.
            nc.vector.tensor_copy(qT[:, :, j * P:(j + 1) * P], tr_ps)

        # ================= phase B group =================================
        # The denominator tail of group g is emitted after the first chunk
        # of group g+1 so its PE ones-matmul never head-blocks the queue
        # waiting for group g's last P_sum accumulation on DVE.
        pending_tail = []

        def flush_tail():
            while pending_tail:
                pending_tail.pop(0)()

        def emit_B(t, h):
            g = h // NREP
            nch = 4 * (t + 1)
            # per-chunk out-proj filler only in B(3) (no A chunks left there)
            cfill_at = set(range(1, nch)) if t == 3 else set()
            ps_y = pps.tile([P, 512], F32, tag="y", bufs=2, name=f"psy{t}_{h}")
            p_sum = pspool.tile([P, 512], BF16, tag="ps")
            # Software-pipelined by one chunk: the y matmul for chunk c is
            # emitted AFTER the score matmul for chunk c+1, so the PE never
            # head-blocks on exp(c) with nothing in front of it.
            prev_y = None
            for c in range(nch):
                o = c * P - t * 512
                col0 = max(o, 0)
                ps_sc = pps.tile([P, 512], F32, tag="sc", bufs=2, name=f"sc{t}_{h}_{c}")
                nc.tensor.matmul(
                    ps_sc[:, col0:512], kT[:, g, c * P:(c + 1) * P],
                    qT[:, h, t * 512 + col0:(t + 1) * 512],
                    start=True, stop=True)
                pt = ptpool.tile([P, 512], BF16, tag="pt")
                nc.scalar.activation(pt[:, col0:512], ps_sc[:, col0:512], AF.Exp)
                if o >= 0:
                    # zero the upper (non-causal) triangle of the diag block
                    nc.gpsimd.affine_select(
                        out=pt[:, col0:col0 + P], in_=pt[:, col0:col0 + P],
                        compare_op=mybir.AluOpType.is_ge, fill=0.0,
                        base=0, pattern=[[1, P]], channel_multiplier=-1)
                if c == 0:
                    nc.vector.tensor_copy(p_sum, pt)
                else:
                    nc.vector.tensor_add(p_sum[:, col0:512], p_sum[:, col0:512],
                                         pt[:, col0:512])
                if prev_y is not None:
                    prev_y()
                if c in cfill_at:
                    fill_c(3)

                def make_y(c=c, col0=col0, pt=pt):
                    def y():
                        nc.tensor.matmul(ps_y[:, col0:512],
                                         v_sb[:, c, g * P:(g + 1) * P],
                                         pt[:, col0:512],
                                         start=(c == 0), stop=(c == nch - 1))
                    return y

                prev_y = make_y()
                if c == 0:
                    flush_tail()
            prev_y()

            def tail():
                # denominator: one ones-matmul, reciprocal, Pool broadcast
                ps_den = pps.tile([P, 512], F32, tag="sc", bufs=2,
                                  name=f"den{t}_{h}")
                nc.tensor.matmul(ps_den[0:1, :], ones_col, p_sum,
                                 start=True, stop=True)
                rc = rpool.tile([1, 512], F32, tag="rc")
                nc.vector.reciprocal(rc, ps_den[0:1, :])
                rden = rpool.tile([P, 512], F32, tag="rd")
                nc.gpsimd.partition_broadcast(rden, rc)
                nc.vector.tensor_mul(yT[:, h, t * 512:(t + 1) * 512], ps_y, rden)

            pending_tail.append(tail)

        # ================= phase C chunk =================================
        # Two flavors:
        #  - emit_C (j<8): whole token-chunk at a B-group boundary; ps_o
        #    rides the sc ring (atomic, so no ring interleaving hazards).
        #  - gen_C (j>=8): fine-grained generator driven from inside B(3)
        #    chunk loops (3 matmuls per B chunk).  Phase A is finished by
        #    then, so ps_o reuses the freed kv PSUM bank.
        def emit_C(j):
            for ct in range(C // 512):
                ps_o = pps.tile([P, 512], F32, tag="sc", bufs=2, name=f"pso{j}_{ct}")
                for hc in range(HG):
                    nc.tensor.matmul(
                        ps_o, yT[:, hc, j * P:(j + 1) * P], wp_sb[:, ct, hc, :],
                        start=(hc == 0), stop=(hc == HG - 1))
                ob = opool.tile([P, 512], F32, tag="ob")
                nc.vector.tensor_copy(ob, ps_o)
                nc.sync.dma_start(out[j * P:(j + 1) * P, ct * 512:(ct + 1) * 512], ob)

        def gen_C(j):
            for ct in range(C // 512):
                # in the drain (j>=12) alternate between the freed kv bank
                # and the sc ring so each ob-copy overlaps the next block's
                # matmuls; interleaved blocks (j<12) stay off the sc ring
                tag = "kv" if (j < 12 or ct % 2 == 0) else "sc"
                bufs = 1 if tag == "kv" else 2
                ps_o = pps.tile([P, 512], F32, tag=tag, bufs=bufs,
                                name=f"pso{j}_{ct}")
                for hc in range(HG):
                    nc.tensor.matmul(
                        ps_o, yT[:, hc, j * P:(j + 1) * P], wp_sb[:, ct, hc, :],
                        start=(hc == 0), stop=(hc == HG - 1))
                    yield
                ob = opool.tile([P, 512], F32, tag="ob")
                nc.vector.tensor_copy(ob, ps_o)
                nc.sync.dma_start(out[j * P:(j + 1) * P, ct * 512:(ct + 1) * 512], ob)
                yield

        c_queue = []

        def fill_c(units):
            while units > 0 and c_queue:
                try:
                    next(c_queue[0])
                    units -= 1
                except StopIteration:
                    c_queue.pop(0)

        # ================= interleaved emission ==========================
        for j in range(4):
            emit_A(j)
        # wp weights: needed from the first C chunk (~mid-kernel)
        wpr = wp.rearrange("(hc p) (ct cc) -> p ct hc cc", p=P, cc=512)
        for ct in range(C // 512):
            nc.scalar.dma_start(wp_sb[:, ct, :, :], wpr[:, ct, :, :])

        # B(t) interleaved with remaining A chunks and C chunks:
        #   B(0) + A(4..7);  B(1) + A(8..11) + C(0..3);
        #   B(2) + A(12..15) + C(4..7);  B(3) + C(8..11) fine-grained;
        #   tail C(12..15)
        fill = {
            0: {0: [("A", 4)], 2: [("A", 5)], 4: [("A", 6)], 6: [("A", 7)]},
            1: {0: [("A", 8)], 1: [("C", 0)], 2: [("A", 9)], 3: [("C", 1)],
                4: [("A", 10)], 5: [("C", 2)], 6: [("A", 11)], 7: [("C", 3)]},
            2: {0: [("A", 12)], 1: [("C", 4)], 2: [("A", 13)], 3: [("C", 5)],
                4: [("A", 14)], 5: [("C", 6)], 6: [("A", 15)], 7: [("C", 7)]},
            3: {0: [("G", 8)], 2: [("G", 9)], 4: [("G", 10)], 6: [("G", 11)]},
        }
        for t in range(4):
            for h in range(HG):
                emit_B(t, h)
                for kind, idx in fill[t].get(h, []):
                    if kind == "A":
                        emit_A(idx)
                    elif kind == "C":
                        emit_C(idx)
                    else:
                        c_queue.append(gen_C(idx))
        flush_tail()
        for j in range(12, 16):
            c_queue.append(gen_C(j))
        fill_c(10 ** 6)

    nc.compile()
    return nc


_NC_CACHE = []


def _get_prog():
    if not _NC_CACHE:
        _NC_CACHE.append(_build())
    return _NC_CACHE[0]


def _make_in_maps(inputs):
    x, cos, sin = inputs["x"], inputs["cos"], inputs["sin"]
    wq, wk, wv, wproj = inputs["wq"], inputs["wk"], inputs["wv"], inputs["wproj"]
    bf = ml_dtypes.bfloat16
    cos2 = np.ascontiguousarray(cos.reshape(T, D // 2)).astype(bf)
    sin2 = np.ascontiguousarray(sin.reshape(T, D // 2)).astype(bf)
    in_maps = []
    for core in range(8):
        b, g = core // 2, core % 2
        qs = slice(g * QC, (g + 1) * QC)
        ks = slice(g * KC, (g + 1) * KC)
        in_maps.append({
            "xt": np.ascontiguousarray(x[b].T).astype(bf),
            "wq": np.ascontiguousarray(wq[:, qs]).astype(bf),
            "wkv": np.ascontiguousarray(np.hstack([wk[:, ks], wv[:, ks]])).astype(bf),
            "wp": np.ascontiguousarray(wproj[qs, :]).astype(bf),
            "cosd": cos2,
            "sind": sin2,
        })
    return in_maps


def kernel(x, cos, sin, wq, wk, wv, wproj):
    nc = _get_prog()
    in_maps = _make_in_maps(dict(x=x, cos=cos, sin=sin, wq=wq, wk=wk, wv=wv, wproj=wproj))
    res = run_bass_kernel_spmd(nc, in_maps, core_ids=list(range(8))).results
    outp = np.empty((B, T, C), np.float32)
    for b in range(B):
        outp[b] = res[2 * b]["out"] + res[2 * b + 1]["out"]
    return outp
